# revision 2
# baseline (speedup 1.0000x reference)
"""Normalized Walsh-Hadamard transform over the last dim of x: (16384, 4096) fp32.

Shard rows across 8 NeuronCores (2048 rows each; rows independent).

Feature split f = 128a + 32c + d (a,d in [0,32), c in [0,4)):
H4096 = H32^(a) x H4^(c) x H32^(d).  Rows are interleaved mod 4 so the
SBUF partition index p = 32*(r%4) + a corresponds to a UNIFORM 128-float
stride in DRAM; the DMA access pattern is then 3-dim with 512 B
contiguous descriptors (full SDMA line rate ~353 GB/s/core vs ~87 for
the 128 B-descriptor layout in this environment).

Per 128-row chunk (16 chunks/core), four 32-row compute slices:
  DMA-in   V[(rm,a), (rq,c,d)] = x[r0+4rq+rm, 128a+32c+d]  SWDGE fp32->fp32r
  stage A  16 accumulating matmuls (c,c' sliced, weights +-kron(I4,H32))
           -> psA[(rm,a'), (c',rq,d)]   [the H4 factor is folded into
           stage A; psA is c'-outer so each matmul output stays inside
           one 2 KB PSUM bank]
  T1       DVE 32x32 block transpose psA -> w0[(rm,d), (c',rq,a')]
  cast     ACT copy w0 -> w1 fp32r, block-permuted to (rq,c',a') via a
           strided input view (BIR requires fp32r-rounded producers for
           fp32r matmuls, so this copy is mandatory anyway)
  stage B  2x512-col matmuls, weight kron(I4,H32)/64 -> psB[(rm,d'), ..]
  T2       DVE 32x32 block transpose psB -> v3[(rm,a'), (rq,c',d')]
  DMA-out  y[r0+4rq+rm, 128a'+32c'+d'] = v3                HWDGE (sync)

Matmul weights are exact in fp32r (+-1, +-1/64); the only rounding is
the TensorE fp32r 11-bit-mantissa data path (~1.4e-4 rel err).
"""
import sys

if "/opt/trn_rl_repo" not in sys.path:
    sys.path.insert(0, "/opt/trn_rl_repo")

import numpy as np

N_CORES = 8
NF = 4096
ROWS_TOTAL = 16384
ROWS_PER_CORE = ROWS_TOTAL // N_CORES


def _hadamard(n):
    h = np.array([[1.0]], dtype=np.float64)
    while h.shape[0] < n:
        h = np.block([[h, h], [h, -h]])
    return h


_H4 = _hadamard(4)


def make_consts():
    w = np.kron(np.eye(4), _hadamard(32)).astype(np.float32)
    return w, -w, w / 64.0


def build_kernel(rows_per_core=ROWS_PER_CORE, chunk_rows=128, reps=1,
                 in_eng="gpsimd", out_eng="sync", vin_bufs=4, w_bufs=4,
                 v3_bufs=4, ps_bufs=2, out_split=1, unroll=None):
    import concourse.tile as tile
    from concourse import bacc, mybir

    assert rows_per_core % chunk_rows == 0 and chunk_rows % 64 == 0
    nchunks = rows_per_core // chunk_rows
    F = chunk_rows * 32          # free elems per partition per chunk
    nh = F // 1024               # 32-row compute slices per chunk
    if unroll is None:
        unroll = 3 if reps % 3 == 0 else 1
    f32, f32r = mybir.dt.float32, mybir.dt.float32r

    nc = bacc.Bacc("TRN2", target_bir_lowering=False, debug=False)
    x_d = nc.dram_tensor("x", [rows_per_core, NF], f32, kind="ExternalInput")
    wp_d = nc.dram_tensor("wp", [128, 128], f32r, kind="ExternalInput")
    wm_d = nc.dram_tensor("wm", [128, 128], f32r, kind="ExternalInput")
    wb_d = nc.dram_tensor("wb", [128, 128], f32r, kind="ExternalInput")
    y_d = nc.dram_tensor("y", [rows_per_core, NF], f32, kind="ExternalOutput")

    def io_ap(t, r0, rows):
        return t.ap()[r0:r0 + rows, :].rearrange(
            "(rq rm) (a cd) -> (rm a) rq cd", rm=4, cd=128)

    with tile.TileContext(nc) as tc:
        with (
            tc.tile_pool(name="consts", bufs=1) as cpool,
            tc.tile_pool(name="vin", bufs=vin_bufs) as vpool,
            tc.tile_pool(name="w0", bufs=w_bufs) as w0pool,
            tc.tile_pool(name="w1", bufs=w_bufs) as w1pool,
            tc.tile_pool(name="v3", bufs=v3_bufs) as v3pool,
            tc.tile_pool(name="psA", bufs=ps_bufs, space="PSUM") as psA,
            tc.tile_pool(name="psB", bufs=ps_bufs, space="PSUM") as psB,
        ):
            wp = cpool.tile([128, 128], f32r)
            wm = cpool.tile([128, 128], f32r)
            wb = cpool.tile([128, 128], f32r)
            nc.sync.dma_start(wp[:], wp_d.ap())
            nc.sync.dma_start(wm[:], wm_d.ap())
            nc.sync.dma_start(wb[:], wb_d.ap())
            oute = getattr(nc, out_eng)

            def body(_it=None):
                for ci in range(nchunks):
                    r0 = ci * chunk_rows
                    if in_eng == "gpsimd":
                        # SWDGE casts fp32 -> fp32r during the transfer
                        v = vpool.tile([128, F], f32r, tag="v")
                        nc.gpsimd.dma_start(
                            v[:].rearrange("p (rq cd) -> p rq cd", cd=128),
                            io_ap(x_d, r0, chunk_rows))
                    else:
                        # HWDGE load (no cast) + ACT cast copy
                        v0 = vpool.tile([128, F], f32, tag="v0")
                        getattr(nc, in_eng).dma_start(
                            v0[:].rearrange("p (rq cd) -> p rq cd", cd=128),
                            io_ap(x_d, r0, chunk_rows))
                        v = vpool.tile([128, F], f32r, tag="v")
                        nc.scalar.copy(v[:], v0[:])
                    v3 = v3pool.tile([128, F], f32, tag="v3")
                    for h in range(nh):
                        mv = v[:, h * 1024:(h + 1) * 1024].rearrange(
                            "p (rq c d) -> p c rq d", c=4, d=32)
                        # psA physical layout (c', rq, d): each c' slice is
                        # a contiguous 256 cols inside one 2KB PSUM bank (a
                        # matmul output may not cross bank boundaries)
                        pa = psA.tile([128, 1024], f32, tag="pa")
                        pao = pa[:].rearrange("p (c rq d) -> p c rq d",
                                              c=4, d=32)
                        for cp in range(4):
                            for c in range(4):
                                w = wp if _H4[c, cp] > 0 else wm
                                nc.tensor.matmul(
                                    pao[:, cp], w[:], mv[:, c],
                                    start=(c == 0), stop=(c == 3))
                        w0 = w0pool.tile([128, 1024], f32, tag="w0")
                        nc.vector.transpose(w0[:], pa[:])
                        # w0 layout (c', rq, a'); the mandatory fp32->fp32r
                        # cast copy also reorders blocks to (rq, c', a')
                        w1 = w1pool.tile([128, 1024], f32r, tag="w1")
                        nc.scalar.copy(
                            w1[:], w0[:].rearrange("p (c rq a) -> p rq c a",
                                                   c=4, a=32))
                        pb = psB.tile([128, 1024], f32, tag="pb")
                        for n in (0, 512):
                            nc.tensor.matmul(pb[:, n:n + 512], wb[:],
                                             w1[:, n:n + 512],
                                             start=True, stop=True)
                        nc.vector.transpose(v3[:, h * 1024:(h + 1) * 1024],
                                            pb[:])
                    srows = chunk_rows // out_split
                    sf = F // out_split
                    for s in range(out_split):
                        oute.dma_start(
                            io_ap(y_d, r0 + s * srows, srows),
                            v3[:, s * sf:(s + 1) * sf].rearrange(
                                "p (rq cd) -> p rq cd", cd=128))

            if reps == 1:
                body()
            else:
                # unroll>1 amortizes the For_i all-engine back-edge
                # barrier over several kernel executions
                assert reps % unroll == 0
                with tc.For_i(0, reps // unroll, 1) as it:
                    for _ in range(unroll):
                        body(it)

    nc.compile()
    return nc


def kernel(x):
    from concourse.bass_utils import run_bass_kernel_spmd

    x = np.asarray(x, dtype=np.float32)
    assert x.shape == (ROWS_TOTAL, NF)
    nc = build_kernel()
    wp, wm, wb = make_consts()
    shards = x.reshape(N_CORES, ROWS_PER_CORE, NF)
    in_maps = [
        {"x": np.ascontiguousarray(shards[i]), "wp": wp, "wm": wm, "wb": wb}
        for i in range(N_CORES)
    ]
    res = run_bass_kernel_spmd(nc, in_maps, core_ids=list(range(N_CORES)))
    y = np.concatenate([res.results[i]["y"] for i in range(N_CORES)], axis=0)
    return np.asarray(y, dtype=np.float32)
